# revision 29
# baseline (speedup 1.0000x reference)
"""DSFourierAttention Trainium2 kernel (restructured mega-wave version).

Math (per (b, h) slice, validated vs the jax reference):
    qf = rfft(q, ortho) etc. as dense DFT matmuls (Fre/Fim [L, X], X = L//2+1)
    qk_T[y, x] = sum_e (kfr qfr + kfi qfi)       (stacked [re; im] K=128 matmul)
    im_T[y, x] = sum_e (-kfi qfr + kfr qfi)      (kf_swap = [-kfi; kfr])
    p = exp(sqrt(re^2 + im^2))                   (no max subtraction; |qk| <= ~5)
    qkv_T[x, e] = (p^T @ [vfr | vfi | ones]) / colsum   (ones col gives colsum)
    out[l, e] = Gre^T @ qkvr + Gim^T @ qkvi      (irfft weights w = [1, 2.., 2, 1])
    out = out * tau[b] + delta[b, l]

Restructure vs baseline: one mega-wave per batch (all QK -> all sqrt ->
all exp, 2 ACT table loads), fused pair squares from [128,2,512] PSUM
pair tiles, batched ragged-column handling via a shared PSUM slot tile,
wide per-head sqrt/exp instructions (in-place, fp16 u), gre/gim
preloaded once.

Sharding: batch-parallel, 2 batches per core across 8 cores.
"""

import os
import sys

import numpy as np

for _p in ("/opt/trn_rl_repo", "/root/.axon_site/_ro/trn_rl_repo"):
    if os.path.isdir(_p) and _p not in sys.path:
        sys.path.insert(0, _p)

import ml_dtypes  # noqa: E402
import concourse.bass as bass  # noqa: E402
import concourse.tile as tile  # noqa: E402
from concourse import bacc, mybir  # noqa: E402
from concourse.bass_utils import run_bass_kernel_spmd  # noqa: E402

B, L, H, E = 16, 1024, 8, 64
X = L // 2 + 1          # 513 rfft bins
XP = X + 1              # padded to 514
NCORES = 8
BL = B // NCORES        # 2 batches per core
NLC = L // 128          # 8 l-chunks
NYC = 4                 # full 128-row y chunks (y=512 handled as ragged row)
NXC = 4                 # full 128-row x chunks

F32 = mybir.dt.float32
F16 = mybir.dt.float16
BF16 = mybir.dt.bfloat16
AF = mybir.ActivationFunctionType

LAST_RESULT = None

# rag slot map (per batch), ONE single-bank PSUM tile [128, 88, 2]:
#   0..31   x=512 col of qk, re, slot 4h+yc      40..71  same, im
#   32..39  corner qk[512,512], re, slot 32+h    72..79  same, im
#   80..87  FFT bin-512 re for (hp, t) at 80+2hp+t (imag bin 512 is 0)
NRAGX = 88


def _consts():
    l = np.arange(L)
    xs = np.arange(X)
    ang = 2.0 * np.pi * np.outer(l, xs) / L          # [L, X]
    fre = np.zeros((L, XP), ml_dtypes.bfloat16)
    fim = np.zeros((L, XP), ml_dtypes.bfloat16)
    fre[:, :X] = (np.cos(ang) / np.sqrt(L)).astype(ml_dtypes.bfloat16)
    fim[:, :X] = (-np.sin(ang) / np.sqrt(L)).astype(ml_dtypes.bfloat16)
    w = np.full(X, 2.0)
    w[0] = 1.0
    w[-1] = 1.0
    gre = (w[:, None] * np.cos(ang.T) / np.sqrt(L)).astype(ml_dtypes.bfloat16)
    gim = (w[:, None] * -np.sin(ang.T) / np.sqrt(L)).astype(ml_dtypes.bfloat16)
    return fre, fim, gre, gim


def build_module(bl=BL, compile=True):
    from concourse.alu_op_type import AluOpType

    nc = bacc.Bacc("TRN2", target_bir_lowering=False, debug=False,
                   num_devices=NCORES)

    qd = nc.dram_tensor("qd", [bl, L, H, E], BF16, kind="ExternalInput").ap()
    kd = nc.dram_tensor("kd", [bl, L, H, E], BF16, kind="ExternalInput").ap()
    vd = nc.dram_tensor("vd", [bl, L, H, E], BF16, kind="ExternalInput").ap()
    taud = nc.dram_tensor("taud", [bl, 1], F32, kind="ExternalInput").ap()
    deltad = nc.dram_tensor("deltad", [bl, L], F32, kind="ExternalInput").ap()
    fred = nc.dram_tensor("fred", [L, XP], BF16, kind="ExternalInput").ap()
    fimd = nc.dram_tensor("fimd", [L, XP], BF16, kind="ExternalInput").ap()
    gred = nc.dram_tensor("gred", [X, L], BF16, kind="ExternalInput").ap()
    gimd = nc.dram_tensor("gimd", [X, L], BF16, kind="ExternalInput").ap()
    outd = nc.dram_tensor("outd", [bl, L, H, E], F32, kind="ExternalOutput").ap()

    with tile.TileContext(nc) as tc:
        _body(nc, tc, AluOpType, qd, kd, vd, taud, deltad, fred, fimd, gred,
              gimd, outd, bl)
    if compile:
        nc.compile()
    return nc


def _body(nc, tc, OPS, qd, kd, vd, taud, deltad, fred, fimd, gred, gimd, outd,
          bl=BL):
    from contextlib import ExitStack

    ctx = ExitStack()
    with ctx:
        P = {}
        P["consts"] = ctx.enter_context(tc.tile_pool(name="consts", bufs=1))
        P["io"] = ctx.enter_context(tc.tile_pool(name="io", bufs=2))
        P["stg"] = ctx.enter_context(tc.tile_pool(name="stg", bufs=2))
        P["stk"] = ctx.enter_context(tc.tile_pool(name="stk", bufs=2))
        P["u"] = ctx.enter_context(tc.tile_pool(name="u", bufs=1))
        P["sq"] = ctx.enter_context(tc.tile_pool(name="sq", bufs=4))
        P["vfp"] = ctx.enter_context(tc.tile_pool(name="vfp", bufs=2))
        P["qkvp"] = ctx.enter_context(tc.tile_pool(name="qkvp", bufs=5))
        P["ep"] = ctx.enter_context(tc.tile_pool(name="ep", bufs=2))
        P["er"] = ctx.enter_context(tc.tile_pool(name="er", bufs=2))
        # PSUM: pp = [128,2,512] pair tiles (4 banks), ph = [128,512]
        # (2 banks), ragx/ragf slot tiles share the rest.
        P["pp"] = ctx.enter_context(tc.tile_pool(name="pp", bufs=2, space="PSUM"))
        P["ph"] = ctx.enter_context(tc.tile_pool(name="ph", bufs=3, space="PSUM"))
        P["ragx"] = ctx.enter_context(tc.tile_pool(name="ragx", bufs=1, space="PSUM"))

        # ---- constants -------------------------------------------------
        fre_sb = P["consts"].tile([128, NLC, XP], BF16)
        fim_sb = P["consts"].tile([128, NLC, XP], BF16)
        for c in range(NLC):
            nc.gpsimd.dma_start(
                out=fre_sb[:, c, :],
                in_=fred.rearrange("(c p) x -> p c x", p=128)[:, c, :])
            nc.gpsimd.dma_start(
                out=fim_sb[:, c, :],
                in_=fimd.rearrange("(c p) x -> p c x", p=128)[:, c, :])
        gre_sb = P["consts"].tile([128, NXC, L], BF16)
        gim_sb = P["consts"].tile([128, NXC, L], BF16)
        for c in range(NXC):
            nc.gpsimd.dma_start(
                out=gre_sb[:, c, :],
                in_=gred[0:512].rearrange("(c p) l -> p c l", p=128)[:, c, :])
            nc.gpsimd.dma_start(
                out=gim_sb[:, c, :],
                in_=gimd[0:512].rearrange("(c p) l -> p c l", p=128)[:, c, :])
        gre512 = P["consts"].tile([1, L], BF16)
        nc.gpsimd.dma_start(out=gre512[0:1, :], in_=gred[512:513, :])

        C = dict(fre=fre_sb, fim=fim_sb, gre=gre_sb, gim=gim_sb, g512=gre512)

        rag2 = P["ragx"].tile([128, NRAGX, 2], F32, tag="rag2", name="rag2")
        vf0 = _phase_v(nc, 0, vd, C, P)
        f0 = _front(nc, OPS, 0, qd, kd, C, P, rag2)
        for b in range(bl):
            last = b == bl - 1
            u16, er_all = _act_tail(nc, b, P, f0[0], f0[1], split=last)
            if not last:
                vf1 = _phase_v(nc, b + 1, vd, C, P)
                f1 = _front(nc, OPS, b + 1, qd, kd, C, P, rag2)
            _pe_tail(nc, OPS, b, taud, deltad, outd, C, P, vf0, u16, er_all)
            if not last:
                vf0, f0 = vf1, f1


def _phase_v(nc, b, vd, C, P):
    """Load v[b] and compute the transposed FFT into vf_av/v512 tiles."""
    v_sb = P["io"].tile([128, NLC, H * E], BF16, tag="vsb", bufs=1,
                        name=f"vsb{b}")
    for c in range(NLC):
        nc.sync.dma_start(
            out=v_sb[:, c, :],
            in_=vd[b].rearrange("(c p) h e -> p c (h e)", p=128)[:, c, :])

    vf_av = []
    for yc in range(NYC):
        t = P["vfp"].tile([128, H, 132], BF16, tag="vfav", bufs=8,
                          name=f"vfav{b}_{yc}")
        vf_av.append(t)
    v512 = P["vfp"].tile([1, H, 132], BF16, tag="v512", bufs=2,
                         name=f"v512_{b}")

    for yc in range(NYC):
        for part, f_sb in ((0, C["fre"]), (1, C["fim"])):
            ps = P["ph"].tile([128, 512], F32, tag="ph",
                              name=f"psv{b}_{yc}_{part}")
            for c in range(NLC):
                nc.tensor.matmul(ps[:, :],
                                 f_sb[:, c, yc * 128:(yc + 1) * 128],
                                 v_sb[:, c, :],
                                 start=(c == 0), stop=(c == NLC - 1))
            nc.vector.tensor_copy(
                out=vf_av[yc][:, :, 64 * part:64 * part + 64],
                in_=ps[:, :].rearrange("p (h e) -> p h e", h=H))
        nc.vector.memset(vf_av[yc][:, :, 128:129], 1.0)

    # ragged y=512 row of vf (imag is 0)
    ps512 = P["ph"].tile([128, 512], F32, tag="ph", name=f"psv512_{b}")
    for c in range(NLC):
        nc.tensor.matmul(ps512[0:1, 0:512],
                         C["fre"][:, c, 512:513],
                         v_sb[:, c, :],
                         start=(c == 0), stop=(c == NLC - 1))
    nc.vector.tensor_copy(out=v512[0:1, :, 0:64],
                          in_=ps512[0:1, 0:512].rearrange("p (h e) -> p h e", h=H))
    nc.vector.memset(v512[0:1, :, 64:128], 0.0)
    nc.vector.memset(v512[0:1, :, 128:129], 1.0)
    return vf_av, v512


def _batch(nc, tc, OPS, b, bl, qd, kd, vd, taud, deltad, outd,
           C, P, vf_cur):
    vf_av, v512 = vf_cur

    # ---- epilogue scalars -------------------------------------------
    tau_sb = P["ep"].tile([128, 1], F32, tag="tau")
    nc.sync.dma_start(out=tau_sb[:, :],
                      in_=taud[b:b + 1, 0:1].to_broadcast([128, 1]))
    delta_sb = P["ep"].tile([128, NLC], F32, tag="delta")
    nc.sync.dma_start(out=delta_sb[:, :],
                      in_=deltad[b, :].rearrange("(c p) -> p c", p=128))

    ragA = P["ragx"].tile([128, NRAGX, 2], F32, tag="ragA", bufs=1,
                          name=f"ragA{b}")
    ragB = P["ragx"].tile([128, NRAGX, 2], F32, tag="ragB", bufs=1,
                          name=f"ragB{b}")

    # ---- q/k FFT + stacking -----------------------------------------
    qstk = P["stk"].tile([128, H, XP], BF16, tag="qstk", name=f"qstk{b}")
    kstk = P["stk"].tile([128, H, XP], BF16, tag="kstk", name=f"kstk{b}")
    kswp = P["stk"].tile([128, H, XP], BF16, tag="kswp", name=f"kswp{b}")

    for hp in range(4):
        q_hp = P["io"].tile([128, NLC, 128], BF16, tag="qhp",
                            name=f"qhp{b}_{hp}")
        nc.sync.dma_start(
            out=q_hp[:, :, :],
            in_=qd[b, :, 2 * hp:2 * hp + 2, :].rearrange(
                "(c p) h e -> p c (h e)", p=128))
        k_hp = P["io"].tile([128, NLC, 128], BF16, tag="khp",
                            name=f"khp{b}_{hp}")
        nc.sync.dma_start(
            out=k_hp[:, :, :],
            in_=kd[b, :, 2 * hp:2 * hp + 2, :].rearrange(
                "(c p) h e -> p c (h e)", p=128))

        for t, src, scale in ((0, q_hp, 0.125), (1, k_hp, 1.0)):
            ps = P["pp"].tile([128, 2, 512], F32, tag="pp",
                              name=f"psf{b}_{hp}_{t}")
            rfs = 40 + 2 * hp + t
            rfA = ragA[:, rfs, :]
            rfB = ragB[:, rfs, :]
            for c in range(NLC):
                lhsT = src[:, c, :]
                nc.tensor.matmul(ps[:, 0, :], lhsT, C["fre"][:, c, 0:512],
                                 start=(c == 0), stop=(c == NLC - 1))
                nc.tensor.matmul(rfA, lhsT, C["fre"][:, c, 512:514],
                                 start=(c == 0), stop=(c == NLC - 1))
                nc.tensor.matmul(ps[:, 1, :], lhsT, C["fim"][:, c, 0:512],
                                 start=(c == 0), stop=(c == NLC - 1))
                nc.tensor.matmul(rfB, lhsT, C["fim"][:, c, 512:514],
                                 start=(c == 0), stop=(c == NLC - 1))
            st_re = P["stg"].tile([128, XP], BF16, tag="stre",
                                  name=f"stre{b}_{hp}_{t}")
            nc.vector.tensor_scalar_mul(out=st_re[:, 0:512],
                                        in0=ps[:, 0, :], scalar1=scale)
            nc.vector.tensor_scalar_mul(out=st_re[:, 512:514],
                                        in0=rfA, scalar1=scale)
            st_im = P["stg"].tile([128, XP], BF16, tag="stim",
                                  name=f"stim{b}_{hp}_{t}")
            nc.vector.tensor_scalar_mul(out=st_im[:, 0:512],
                                        in0=ps[:, 1, :], scalar1=scale)
            nc.vector.tensor_scalar_mul(out=st_im[:, 512:514],
                                        in0=rfB, scalar1=scale)
            if t == 1:
                st_imn = P["stg"].tile([128, XP], BF16, tag="stimn", bufs=1,
                                       name=f"stimn{b}_{hp}")
                nc.vector.tensor_scalar_mul(out=st_imn[:, 0:512],
                                            in0=ps[:, 1, :], scalar1=-1.0)
                nc.vector.tensor_scalar_mul(out=st_imn[:, 512:514],
                                            in0=rfB, scalar1=-1.0)

            for phi in range(2):
                h = 2 * hp + phi
                rows = slice(64 * phi, 64 * phi + 64)
                if t == 0:
                    nc.gpsimd.dma_start(out=qstk[0:64, h, :], in_=st_re[rows, :])
                    nc.gpsimd.dma_start(out=qstk[64:128, h, :], in_=st_im[rows, :])
                else:
                    nc.gpsimd.dma_start(out=kstk[0:64, h, :], in_=st_re[rows, :])
                    nc.gpsimd.dma_start(out=kstk[64:128, h, :], in_=st_im[rows, :])
                    nc.gpsimd.dma_start(out=kswp[0:64, h, :], in_=st_imn[rows, :])
                    nc.gpsimd.dma_start(out=kswp[64:128, h, :], in_=st_re[rows, :])

    # ---- QK + squares ------------------------------------------------
    # u_all[p, h, yc, x] = |qk|^2 -> (in place) |qk| -> exp(|qk|)
    u_all = P["u"].tile([128, H, NYC, 520], F16, tag="uall", name=f"uall{b}")
    u_rag = P["u"].tile([128, 2, 520], F16, tag="urag", name=f"urag{b}")
    nc.vector.memset(ragA[:, 0:40, :], 0.0)
    nc.vector.memset(ragB[:, 0:40, :], 0.0)

    for h in range(H):
        for yc in range(NYC):
            ycs = slice(yc * 128, (yc + 1) * 128)
            pq = P["pp"].tile([128, 2, 512], F32, tag="pp",
                              name=f"psqk{b}_{h}_{yc}")
            s = 4 * h + yc
            nc.tensor.matmul(pq[:, 0, :], kstk[:, h, ycs],
                             qstk[:, h, 0:512], start=True, stop=True)
            nc.tensor.matmul(ragA[:, s, :], kstk[:, h, ycs],
                             qstk[:, h, 512:514], start=True, stop=True)
            nc.tensor.matmul(pq[:, 1, :], kswp[:, h, ycs],
                             qstk[:, h, 0:512], start=True, stop=True)
            nc.tensor.matmul(ragB[:, s, :], kswp[:, h, ycs],
                             qstk[:, h, 512:514], start=True, stop=True)
            sq = P["sq"].tile([128, 2, 512], F16, tag="sq",
                              name=f"sq{b}_{h}_{yc}")
            nc.scalar.square(out=sq[:, :, :], in_=pq[:, :, :])
            nc.vector.tensor_tensor(out=u_all[:, h, yc, 0:512],
                                    in0=sq[:, 0, :], in1=sq[:, 1, :],
                                    op=OPS.add)

    # ---- ragged y=512 rows (ph pool is idle during the QK phase) -----
    # head h's row lands at partition 32*(h%4) of group h//4
    for t_grp in range(2):
        rr_re = P["ph"].tile([128, 512], F32, tag="ph",
                             name=f"ragrowre{b}_{t_grp}")
        rr_im = P["ph"].tile([128, 512], F32, tag="ph",
                             name=f"ragrowim{b}_{t_grp}")
        nc.vector.memset(rr_re[:, :], 0.0)
        nc.vector.memset(rr_im[:, :], 0.0)
        for hh in range(4):
            h = 4 * t_grp + hh
            bp = 32 * hh
            nc.tensor.matmul(rr_re[bp:bp + 1, :], kstk[:, h, 512:513],
                             qstk[:, h, 0:512], start=True, stop=True,
                             tile_position=(0, bp))
            nc.tensor.matmul(ragA[bp:bp + 1, 32 + h, :],
                             kstk[:, h, 512:513],
                             qstk[:, h, 512:514], start=True, stop=True,
                             tile_position=(0, bp))
            nc.tensor.matmul(rr_im[bp:bp + 1, :], kswp[:, h, 512:513],
                             qstk[:, h, 0:512], start=True, stop=True,
                             tile_position=(0, bp))
            nc.tensor.matmul(ragB[bp:bp + 1, 32 + h, :],
                             kswp[:, h, 512:513],
                             qstk[:, h, 512:514], start=True, stop=True,
                             tile_position=(0, bp))
        sqr = P["sq"].tile([128, 2, 512], F16, tag="sq", bufs=3,
                           name=f"sqrag{b}_{t_grp}")
        nc.scalar.square(out=sqr[:, 0, :], in_=rr_re[:, :])
        nc.scalar.square(out=sqr[:, 1, :], in_=rr_im[:, :])
        nc.vector.tensor_tensor(out=u_rag[:, t_grp, 0:512],
                                in0=sqr[:, 0, :], in1=sqr[:, 1, :],
                                op=OPS.add)

    # ---- ragged x=512 columns (batched) ------------------------------
    sqA = P["sq"].tile([128, 40, 2], F32, tag="sqx", name=f"sqA{b}")
    sqB = P["sq"].tile([128, 40, 2], F32, tag="sqx", name=f"sqB{b}")
    nc.scalar.square(out=sqA[:, :, :], in_=ragA[:, 0:40, :])
    nc.scalar.square(out=sqB[:, :, :], in_=ragB[:, 0:40, :])
    # u_all[:, h, yc, 512] = re^2 + im^2 from slot 4h+yc
    nc.vector.tensor_tensor(
        out=u_all[:, :, :, 512].rearrange("p h y -> p (h y)"),
        in0=sqA[:, 0:32, 0], in1=sqB[:, 0:32, 0], op=OPS.add)
    # corners -> u_rag[:, t, 512]
    ucorn = P["er"].tile([128, H], F32, tag="ucorn", name=f"ucorn{b}")
    nc.vector.tensor_tensor(out=ucorn[:, :], in0=sqA[:, 32:40, 0],
                            in1=sqB[:, 32:40, 0], op=OPS.add)
    with nc.allow_low_precision(reason="4-term corner sum, values <= ~170"):
        nc.vector.tensor_reduce(out=u_rag[:, 0, 512:513], in_=ucorn[:, 0:4],
                                axis=mybir.AxisListType.X, op=OPS.add)
        nc.vector.tensor_reduce(out=u_rag[:, 1, 512:513], in_=ucorn[:, 4:8],
                                axis=mybir.AxisListType.X, op=OPS.add)

    # ---- sqrt (one table load), then exp (one table load) ------------
    for h in range(H):
        nc.scalar.sqrt(out=u_all[:, h, :, 0:X], in_=u_all[:, h, :, 0:X])
    nc.scalar.sqrt(out=u_rag[:, :, 0:X], in_=u_rag[:, :, 0:X])
    u16 = u_all.bitcast(BF16)
    for h in range(H):
        nc.scalar.activation(out=u16[:, h, :, 0:X], in_=u_all[:, h, :, 0:X],
                             func=AF.Exp)
    ur16 = u_rag.bitcast(BF16)
    nc.scalar.activation(out=ur16[:, :, 0:X], in_=u_rag[:, :, 0:X],
                         func=AF.Exp)

    # rag rows to partition 0 for the K=1 AV matmuls
    er_all = {}
    for h in range(H):
        bp = 32 * (h % 4)
        er = P["er"].tile([1, X], BF16, tag="er", bufs=8, name=f"er{b}_{h}")
        er_all[h] = er
        nc.sync.dma_start(out=er[0:1, :],
                          in_=ur16[bp:bp + 1, h // 4, 0:X])

    # ---- qkv accumulators + AV --------------------------------------
    qkv_all = [P["qkvp"].tile([128, 2, H, 64], BF16, tag="qkv",
                              name=f"qkv{b}_{xc}")
               for xc in range(NXC)]
    qkv512 = P["qkvp"].tile([1, 2, H, 64], BF16, tag="qkv512", bufs=2,
                            name=f"qkv512_{b}")

    for h in range(H):
        for xc in range(NXC):
            xcs = slice(xc * 128, (xc + 1) * 128)
            ps_av = P["ph"].tile([128, 512], F32, tag="ph",
                                 name=f"psav{b}_{h}_{xc}")
            for yc in range(NYC):
                nc.tensor.matmul(ps_av[:, 0:129],
                                 u16[:, h, yc, xcs],
                                 vf_av[yc][:, h, 0:129],
                                 start=(yc == 0), stop=False)
            nc.tensor.matmul(ps_av[:, 0:129], er_all[h][0:1, xcs],
                             v512[0:1, h, 0:129], start=False, stop=True)
            rc = P["er"].tile([128, 1], F32, tag="rc", bufs=4,
                              name=f"rc{b}_{h}_{xc}")
            nc.vector.reciprocal(out=rc[:, :], in_=ps_av[:, 128:129])
            nc.vector.tensor_scalar_mul(
                out=qkv_all[xc][:, :, h, :],
                in0=ps_av[:, 0:128].rearrange("p (t e) -> p t e", t=2),
                scalar1=rc[:, 0:1])
        # ragged x = 512 row of qkv
        ps_a1 = P["ph"].tile([128, 512], F32, tag="ph", name=f"psa1{b}_{h}")
        for yc in range(NYC):
            nc.tensor.matmul(ps_a1[0:1, 0:129],
                             u16[:, h, yc, 512:513],
                             vf_av[yc][:, h, 0:129],
                             start=(yc == 0), stop=False)
        nc.tensor.matmul(ps_a1[0:1, 0:129], er_all[h][0:1, 512:513],
                         v512[0:1, h, 0:129], start=False, stop=True)
        rc1 = P["er"].tile([1, 1], F32, tag="rc1", bufs=2, name=f"rc1{b}_{h}")
        nc.vector.reciprocal(out=rc1[0:1, :], in_=ps_a1[0:1, 128:129])
        nc.vector.tensor_scalar_mul(
            out=qkv512[0:1, :, h, :],
            in0=ps_a1[0:1, 0:128].rearrange("p (t e) -> p t e", t=2),
            scalar1=rc1[0:1, 0:1])

    # next batch's independent v-FFT emitted before the iFFT so the PE
    # has work while this batch's softmax/AV tail drains
    vf_next = None
    if b + 1 < bl:
        vf_next = _phase_v(nc, b + 1, vd, C, P)

    # ---- iFFT + epilogue --------------------------------------------
    for lc in range(NLC):
        lcs = slice(lc * 128, (lc + 1) * 128)
        ps_o = P["ph"].tile([128, 512], F32, tag="ph", name=f"pso{b}_{lc}")
        for xc in range(NXC):
            nc.tensor.matmul(ps_o[:, 0:512],
                             C["gre"][:, xc, lcs],
                             qkv_all[xc][:, 0, :, :],
                             start=(xc == 0), stop=False)
            nc.tensor.matmul(ps_o[:, 0:512],
                             C["gim"][:, xc, lcs],
                             qkv_all[xc][:, 1, :, :],
                             start=False, stop=False)
        nc.tensor.matmul(ps_o[:, 0:512],
                         C["g512"][0:1, lc * 128:(lc + 1) * 128],
                         qkv512[0:1, 0, :, :],
                         start=False, stop=True)
        out_t = P["ep"].tile([128, 512], F32, tag="outsb", name=f"out{b}_{lc}")
        nc.vector.tensor_scalar(out=out_t[:, :], in0=ps_o[:, 0:512],
                                scalar1=tau_sb[:, 0:1],
                                scalar2=delta_sb[:, lc:lc + 1],
                                op0=OPS.mult, op1=OPS.add)
        nc.sync.dma_start(
            out=outd[b, lc * 128:(lc + 1) * 128, :, :].rearrange(
                "l h e -> l (h e)"),
            in_=out_t[:, :])
    return vf_next


_BUILT = None
_CONSTS = None


def _get_built():
    global _BUILT, _CONSTS
    if _BUILT is None:
        _BUILT = build_module()
        _CONSTS = _consts()
    return _BUILT, _CONSTS


def kernel(q, k, v, mask, tau, delta):
    global LAST_RESULT
    nc, (fre, fim, gre, gim) = _get_built()
    q = np.ascontiguousarray(np.asarray(q, dtype=np.float32)).astype(ml_dtypes.bfloat16)
    k = np.ascontiguousarray(np.asarray(k, dtype=np.float32)).astype(ml_dtypes.bfloat16)
    v = np.ascontiguousarray(np.asarray(v, dtype=np.float32)).astype(ml_dtypes.bfloat16)
    tau = np.ascontiguousarray(np.asarray(tau, dtype=np.float32))
    delta = np.ascontiguousarray(np.asarray(delta, dtype=np.float32))

    in_maps = []
    for i in range(NCORES):
        sl = slice(i * BL, (i + 1) * BL)
        in_maps.append({
            "qd": np.ascontiguousarray(q[sl]),
            "kd": np.ascontiguousarray(k[sl]),
            "vd": np.ascontiguousarray(v[sl]),
            "taud": np.ascontiguousarray(tau[sl]),
            "deltad": np.ascontiguousarray(delta[sl]),
            "fred": fre, "fimd": fim, "gred": gre, "gimd": gim,
        })
    res = run_bass_kernel_spmd(nc, in_maps, core_ids=list(range(NCORES)))
    LAST_RESULT = res
    out = np.concatenate([res.results[i]["outd"] for i in range(NCORES)], axis=0)
    return out.astype(np.float32)
